# revision 1
# baseline (speedup 1.0000x reference)
"""GraphSAGE-style 2-layer GNN encoder on 8 TRN2 NeuronCores.

Sharding: nodes partitioned by destination across 8 cores (6250 each).
Each core owns the incoming edges of its dst shard; source features are
gathered from a replicated table (x for layer 1, all-gathered h1 for
layer 2). Segment-sum is computed as a sequence of one-hot selection
matmuls: for each 128-dst window, chunks of 128 edges are gathered
([128 edges, 128 feat]) and accumulated into PSUM via S^T @ G where
S[e, j] = (dst_rel(e) == j).
"""

import numpy as np

import concourse.bacc as bacc
import concourse.bass as bass
import concourse.mybir as mybir
import concourse.tile as tile
from concourse.bass_utils import run_bass_kernel_spmd
from concourse.masks import make_identity

N_NODES = 50000
N_EDGES = 800000
D = 128
NCORES = 8
NLOC = N_NODES // NCORES  # 6250
NWIN = (NLOC + 127) // 128  # 49
NPAD = NWIN * 128  # 6272
EPS = 1e-7
PAD_DST = 200.0  # sentinel dst_rel for padding edges (no one-hot match)

F32 = mybir.dt.float32
I32 = mybir.dt.int32

_cache = {}


def _build(k_chunks: int):
    nc = bacc.Bacc(
        "TRN2", target_bir_lowering=False, debug=False,
        enable_asserts=True, num_devices=NCORES,
    )
    K = k_chunks
    NJ = NWIN * K  # chunk count per layer

    xloc = nc.declare_dram_parameter("xloc", [NPAD, D], F32, isOutput=False)
    xfull = nc.declare_dram_parameter("xfull", [N_NODES, D], F32, isOutput=False)
    srcs = nc.declare_dram_parameter("srcs", [128, NJ], I32, isOutput=False)
    dstr = nc.declare_dram_parameter("dstr", [128, NJ], F32, isOutput=False)
    nnbt = nc.declare_dram_parameter("nnbt", [128, NWIN], F32, isOutput=False)
    w1 = nc.declare_dram_parameter("w1", [2 * D, D], F32, isOutput=False)
    b1 = nc.declare_dram_parameter("b1", [D, 1], F32, isOutput=False)
    w2 = nc.declare_dram_parameter("w2", [2 * D, D], F32, isOutput=False)
    b2 = nc.declare_dram_parameter("b2", [D, 1], F32, isOutput=False)
    out = nc.declare_dram_parameter("out", [NLOC, 2 * D], F32, isOutput=True)

    with tile.TileContext(nc) as tc:
        with (
            tc.tile_pool(name="const", bufs=1) as constp,
            tc.tile_pool(name="meta", bufs=1) as metap,
            tc.tile_pool(name="keep", bufs=1) as keepp,
            tc.tile_pool(name="gp", bufs=6) as gp,
            tc.tile_pool(name="sp", bufs=4) as sp,
            tc.tile_pool(name="winp", bufs=3) as winp,
            tc.tile_pool(name="psA", bufs=2, space="PSUM") as psA,
            tc.tile_pool(name="psB", bufs=2, space="PSUM") as psB,
            tc.tile_pool(name="psC", bufs=2, space="PSUM") as psC,
            tc.tile_pool(name="dram", bufs=1, space="DRAM") as dram,
        ):
            # ---- constants / metadata ----
            ident = constp.tile([128, 128], F32)
            make_identity(nc, ident[:])
            iota_i = constp.tile([128, 128], I32)
            nc.gpsimd.iota(iota_i[:], pattern=[[1, 128]], base=0, channel_multiplier=0)
            iota_f = constp.tile([128, 128], F32)
            nc.vector.tensor_copy(out=iota_f[:], in_=iota_i[:])

            w1t = constp.tile([128, D], F32)
            nc.sync.dma_start(out=w1t[:], in_=w1[0:D, :])
            w1b = constp.tile([128, D], F32)
            nc.sync.dma_start(out=w1b[:], in_=w1[D:2 * D, :])
            w2t = constp.tile([128, D], F32)
            nc.sync.dma_start(out=w2t[:], in_=w2[0:D, :])
            w2b = constp.tile([128, D], F32)
            nc.sync.dma_start(out=w2b[:], in_=w2[D:2 * D, :])
            b1s = constp.tile([128, 1], F32)
            nc.sync.dma_start(out=b1s[:], in_=b1[:, :])
            b2s = constp.tile([128, 1], F32)
            nc.sync.dma_start(out=b2s[:], in_=b2[:, :])

            srcs_sb = metap.tile([128, NJ], I32)
            nc.sync.dma_start(out=srcs_sb[:], in_=srcs[:, :])
            dstr_sb = metap.tile([128, NJ], F32)
            nc.sync.dma_start(out=dstr_sb[:], in_=dstr[:, :])

            nnbt_sb = metap.tile([128, NWIN], F32)
            nc.sync.dma_start(out=nnbt_sb[:], in_=nnbt[:, :])
            recip = metap.tile([128, NWIN], F32)
            nc.vector.tensor_scalar_add(out=recip[:], in0=nnbt_sb[:], scalar1=EPS)
            nc.vector.reciprocal(out=recip[:], in_=recip[:])

            # layer-1 hidden (transposed windows) kept resident for layer 2
            h1T = keepp.tile([128, NPAD], F32)

            # internal DRAM: layer-1 row-major output + all-gathered table
            h1loc = dram.tile([NLOC, D], F32)
            h1full = dram.tile([N_NODES, D], F32, addr_space="Shared")

            def layer(li, table, wt, wb, bs):
                for w in range(NWIN):
                    rows0 = w * 128
                    nrows = min(128, NLOC - rows0)
                    # ---- segment-sum over this window's edge chunks ----
                    acc = psA.tile([128, 128], F32, tag="acc")
                    for k in range(K):
                        j = w * K + k
                        g = gp.tile([128, 128], F32, tag="g")
                        nc.gpsimd.indirect_dma_start(
                            out=g[:], out_offset=None,
                            in_=table[:, :],
                            in_offset=bass.IndirectOffsetOnAxis(
                                ap=srcs_sb[:, j:j + 1], axis=0),
                        )
                        s_t = sp.tile([128, 128], F32, tag="s")
                        nc.vector.tensor_scalar(
                            out=s_t[:], in0=iota_f[:],
                            scalar1=dstr_sb[:, j:j + 1], scalar2=None,
                            op0=mybir.AluOpType.is_equal,
                        )
                        nc.tensor.matmul(
                            acc[:], lhsT=s_t[:], rhs=g[:],
                            start=(k == 0), stop=(k == K - 1),
                        )
                    # ---- mean + transpose ----
                    neigh = winp.tile([128, 128], F32, tag="neigh")
                    nc.scalar.activation(
                        out=neigh[:], in_=acc[:],
                        func=mybir.ActivationFunctionType.Copy,
                        scale=recip[:, w:w + 1],
                    )
                    ntp = psB.tile([128, 128], F32, tag="tp")
                    nc.tensor.transpose(out=ntp[:], in_=neigh[:], identity=ident[:])
                    neighT = winp.tile([128, 128], F32, tag="neighT")
                    nc.vector.tensor_copy(out=neighT[:], in_=ntp[:])

                    # ---- self features (transposed) ----
                    if li == 0:
                        xwin = winp.tile([128, 128], F32, tag="xwin")
                        nc.sync.dma_start(
                            out=xwin[:], in_=xloc[rows0:rows0 + 128, :])
                        # pass-through concat: out[:, D:2D] = x
                        nc.sync.dma_start(
                            out=out[rows0:rows0 + nrows, D:2 * D],
                            in_=xwin[:nrows, :])
                        stp = psB.tile([128, 128], F32, tag="tp")
                        nc.tensor.transpose(out=stp[:], in_=xwin[:], identity=ident[:])
                        selfT = winp.tile([128, 128], F32, tag="selfT")
                        nc.vector.tensor_copy(out=selfT[:], in_=stp[:])
                    else:
                        selfT = h1T[:, rows0:rows0 + 128]

                    # ---- out_T = relu(W_top^T @ selfT + W_bot^T @ neighT + b) ----
                    op = psC.tile([128, 128], F32, tag="op")
                    nc.tensor.matmul(op[:], lhsT=wt[:], rhs=(selfT[:] if li == 0 else selfT),
                                     start=True, stop=False)
                    nc.tensor.matmul(op[:], lhsT=wb[:], rhs=neighT[:],
                                     start=False, stop=True)
                    if li == 0:
                        hT_dst = h1T[:, rows0:rows0 + 128]
                    else:
                        hT_t = winp.tile([128, 128], F32, tag="hTout")
                        hT_dst = hT_t[:]
                    nc.scalar.activation(
                        out=hT_dst, in_=op[:],
                        func=mybir.ActivationFunctionType.Relu,
                        bias=bs[:, :1], scale=1.0,
                    )
                    # ---- row-major copy for gather table / output ----
                    rtp = psB.tile([128, 128], F32, tag="tp")
                    nc.tensor.transpose(out=rtp[:], in_=hT_dst, identity=ident[:])
                    h_rm = winp.tile([128, 128], F32, tag="hrm")
                    nc.vector.tensor_copy(out=h_rm[:], in_=rtp[:])
                    if li == 0:
                        nc.sync.dma_start(
                            out=h1loc[rows0:rows0 + nrows, :], in_=h_rm[:nrows, :])
                    else:
                        nc.sync.dma_start(
                            out=out[rows0:rows0 + nrows, 0:D], in_=h_rm[:nrows, :])

            layer(0, xfull, w1t, w1b, b1s)
            nc.gpsimd.collective_compute(
                "AllGather", mybir.AluOpType.bypass,
                replica_groups=[list(range(NCORES))],
                ins=[h1loc.opt()], outs=[h1full.opt()],
            )
            layer(1, h1full, w2t, w2b, b2s)

    nc.compile()
    return nc


def _prep_core(c, x, edge_src, edge_dst, num_neighbors, K):
    lo = c * NLOC
    m = (edge_dst >= lo) & (edge_dst < lo + NLOC)
    es = edge_src[m].astype(np.int64)
    ed = (edge_dst[m].astype(np.int64) - lo)
    w = ed >> 7
    order = np.argsort(w, kind="stable")
    es, ed, w = es[order], ed[order], w[order]
    counts = np.bincount(w, minlength=NWIN)
    starts = np.cumsum(counts) - counts
    pos = np.arange(len(w)) - np.repeat(starts, counts)
    srcs = np.zeros((NWIN, K * 128), np.int32)
    dstr = np.full((NWIN, K * 128), PAD_DST, np.float32)
    srcs[w, pos] = es
    dstr[w, pos] = (ed & 127).astype(np.float32)
    srcs_sb = np.ascontiguousarray(
        srcs.reshape(NWIN, K, 128).transpose(2, 0, 1).reshape(128, NWIN * K))
    dstr_sb = np.ascontiguousarray(
        dstr.reshape(NWIN, K, 128).transpose(2, 0, 1).reshape(128, NWIN * K))
    nnb = np.ones((NPAD,), np.float32)
    nnb[:NLOC] = num_neighbors[lo:lo + NLOC]
    nnbt = np.ascontiguousarray(nnb.reshape(NWIN, 128).T)
    xl = np.zeros((NPAD, D), np.float32)
    xl[:NLOC] = x[lo:lo + NLOC]
    return srcs_sb, dstr_sb, nnbt, xl


def kernel(x, edge_src, edge_dst, num_neighbors, W1, b1, W2, b2):
    x = np.ascontiguousarray(np.asarray(x, dtype=np.float32))
    edge_src = np.asarray(edge_src, dtype=np.int32)
    edge_dst = np.asarray(edge_dst, dtype=np.int32)
    num_neighbors = np.asarray(num_neighbors, dtype=np.float32)
    W1 = np.ascontiguousarray(np.asarray(W1, dtype=np.float32))
    W2 = np.ascontiguousarray(np.asarray(W2, dtype=np.float32))
    b1 = np.asarray(b1, dtype=np.float32).reshape(D, 1)
    b2 = np.asarray(b2, dtype=np.float32).reshape(D, 1)

    # fixed chunk count across cores/windows (one SPMD program)
    core_id = edge_dst.astype(np.int64) // NLOC
    win_id = core_id * NWIN + ((edge_dst.astype(np.int64) % NLOC) >> 7)
    max_cnt = np.bincount(win_id, minlength=NCORES * NWIN).max()
    K = int((max_cnt + 127) // 128)

    if K not in _cache:
        _cache[K] = _build(K)
    nc = _cache[K]

    in_maps = []
    for c in range(NCORES):
        srcs_sb, dstr_sb, nnbt, xl = _prep_core(
            c, x, edge_src, edge_dst, num_neighbors, K)
        in_maps.append({
            "xloc": xl, "xfull": x, "srcs": srcs_sb, "dstr": dstr_sb,
            "nnbt": nnbt, "w1": W1, "b1": b1, "w2": W2, "b2": b2,
        })

    res = run_bass_kernel_spmd(nc, in_maps, list(range(NCORES)))
    return np.concatenate([res.results[c]["out"] for c in range(NCORES)], axis=0)


# revision 2
# speedup vs baseline: 3096.8985x; 3096.8985x over previous
"""GraphSAGE-style 2-layer GNN encoder on 8 TRN2 NeuronCores.

Sharding: nodes partitioned by destination across 8 cores (6250 each).
Each core owns the incoming edges of its dst shard; source features are
gathered from a replicated table (x for layer 1, all-gathered h1 for
layer 2). Segment-sum is computed as a sequence of one-hot selection
matmuls: for each 128-dst window, chunks of 128 edges are gathered
([128 edges, 128 feat]) and accumulated into PSUM via S^T @ G where
S[e, j] = (dst_rel(e) == j).
"""

import numpy as np

import concourse.bacc as bacc
import concourse.bass as bass
import concourse.mybir as mybir
import concourse.tile as tile
from concourse.bass_utils import run_bass_kernel_spmd
from concourse.masks import make_identity

N_NODES = 50000
N_EDGES = 800000
D = 128
NCORES = 8
NLOC = N_NODES // NCORES  # 6250
NWIN = (NLOC + 127) // 128  # 49
NPAD = NWIN * 128  # 6272
EPS = 1e-7
PAD_DST = 200.0  # sentinel dst_rel for padding edges (no one-hot match)

F32 = mybir.dt.float32
I32 = mybir.dt.int32

_cache = {}


def _build(k_chunks: int):
    nc = bacc.Bacc(
        "TRN2", target_bir_lowering=False, debug=False,
        enable_asserts=True, num_devices=NCORES,
    )
    K = k_chunks
    NJ = NWIN * K  # chunk count per layer

    xloc = nc.declare_dram_parameter("xloc", [NPAD, D], F32, isOutput=False)
    xfull = nc.declare_dram_parameter("xfull", [N_NODES, D], F32, isOutput=False)
    srcs = nc.declare_dram_parameter("srcs", [128, NJ], I32, isOutput=False)
    dstr = nc.declare_dram_parameter("dstr", [128, NJ], F32, isOutput=False)
    nnbt = nc.declare_dram_parameter("nnbt", [128, NWIN], F32, isOutput=False)
    w1 = nc.declare_dram_parameter("w1", [2 * D, D], F32, isOutput=False)
    b1 = nc.declare_dram_parameter("b1", [D, 1], F32, isOutput=False)
    w2 = nc.declare_dram_parameter("w2", [2 * D, D], F32, isOutput=False)
    b2 = nc.declare_dram_parameter("b2", [D, 1], F32, isOutput=False)
    out = nc.declare_dram_parameter("out", [NLOC, 2 * D], F32, isOutput=True)

    with tile.TileContext(nc) as tc:
        with (
            tc.tile_pool(name="const", bufs=1) as constp,
            tc.tile_pool(name="meta", bufs=1) as metap,
            tc.tile_pool(name="keep", bufs=1) as keepp,
            tc.tile_pool(name="gp", bufs=6) as gp,
            tc.tile_pool(name="sp", bufs=4) as sp,
            tc.tile_pool(name="winp", bufs=3) as winp,
            tc.tile_pool(name="psA", bufs=2, space="PSUM") as psA,
            tc.tile_pool(name="psB", bufs=2, space="PSUM") as psB,
            tc.tile_pool(name="psC", bufs=2, space="PSUM") as psC,
            tc.tile_pool(name="dram", bufs=1, space="DRAM") as dram,
        ):
            # ---- constants / metadata ----
            ident = constp.tile([128, 128], F32)
            make_identity(nc, ident[:])
            iota_i = constp.tile([128, 128], I32)
            nc.gpsimd.iota(iota_i[:], pattern=[[1, 128]], base=0, channel_multiplier=0)
            iota_f = constp.tile([128, 128], F32)
            nc.vector.tensor_copy(out=iota_f[:], in_=iota_i[:])

            w1t = constp.tile([128, D], F32)
            nc.sync.dma_start(out=w1t[:], in_=w1[0:D, :])
            w1b = constp.tile([128, D], F32)
            nc.sync.dma_start(out=w1b[:], in_=w1[D:2 * D, :])
            w2t = constp.tile([128, D], F32)
            nc.sync.dma_start(out=w2t[:], in_=w2[0:D, :])
            w2b = constp.tile([128, D], F32)
            nc.sync.dma_start(out=w2b[:], in_=w2[D:2 * D, :])
            b1s = constp.tile([128, 1], F32)
            nc.sync.dma_start(out=b1s[:], in_=b1[:, :])
            b2s = constp.tile([128, 1], F32)
            nc.sync.dma_start(out=b2s[:], in_=b2[:, :])

            srcs_sb = metap.tile([128, NJ], I32)
            nc.sync.dma_start(out=srcs_sb[:], in_=srcs[:, :])
            dstr_sb = metap.tile([128, NJ], F32)
            nc.sync.dma_start(out=dstr_sb[:], in_=dstr[:, :])

            nnbt_sb = metap.tile([128, NWIN], F32)
            nc.sync.dma_start(out=nnbt_sb[:], in_=nnbt[:, :])
            recip = metap.tile([128, NWIN], F32)
            nc.vector.tensor_scalar_add(out=recip[:], in0=nnbt_sb[:], scalar1=EPS)
            nc.vector.reciprocal(out=recip[:], in_=recip[:])

            # layer-1 hidden (transposed windows) kept resident for layer 2
            h1T = keepp.tile([128, NPAD], F32)

            # internal DRAM: layer-1 row-major output + all-gathered table
            h1loc = dram.tile([NLOC, D], F32)
            h1full = dram.tile([N_NODES, D], F32, addr_space="Shared")

            def layer(li, table, wt, wb, bs):
                for w in range(NWIN):
                    rows0 = w * 128
                    nrows = min(128, NLOC - rows0)
                    # ---- segment-sum over this window's edge chunks ----
                    acc = psA.tile([128, 128], F32, tag="acc")
                    for k in range(K):
                        j = w * K + k
                        g = gp.tile([128, 128], F32, tag="g")
                        nc.gpsimd.indirect_dma_start(
                            out=g[:], out_offset=None,
                            in_=table[:, :],
                            in_offset=bass.IndirectOffsetOnAxis(
                                ap=srcs_sb[:, j:j + 1], axis=0),
                        )
                        s_t = sp.tile([128, 128], F32, tag="s")
                        nc.vector.tensor_scalar(
                            out=s_t[:], in0=iota_f[:],
                            scalar1=dstr_sb[:, j:j + 1], scalar2=None,
                            op0=mybir.AluOpType.is_equal,
                        )
                        nc.tensor.matmul(
                            acc[:], lhsT=s_t[:], rhs=g[:],
                            start=(k == 0), stop=(k == K - 1),
                        )
                    # ---- mean + transpose ----
                    neigh = winp.tile([128, 128], F32, tag="neigh")
                    nc.scalar.activation(
                        out=neigh[:], in_=acc[:],
                        func=mybir.ActivationFunctionType.Copy,
                        scale=recip[:, w:w + 1],
                    )
                    ntp = psB.tile([128, 128], F32, tag="tp")
                    nc.tensor.transpose(out=ntp[:], in_=neigh[:], identity=ident[:])
                    neighT = winp.tile([128, 128], F32, tag="neighT")
                    nc.vector.tensor_copy(out=neighT[:], in_=ntp[:])

                    # ---- self features (transposed) ----
                    if li == 0:
                        xwin = winp.tile([128, 128], F32, tag="xwin")
                        nc.sync.dma_start(
                            out=xwin[:], in_=xloc[rows0:rows0 + 128, :])
                        # pass-through concat: out[:, D:2D] = x
                        nc.sync.dma_start(
                            out=out[rows0:rows0 + nrows, D:2 * D],
                            in_=xwin[:nrows, :])
                        stp = psB.tile([128, 128], F32, tag="tp")
                        nc.tensor.transpose(out=stp[:], in_=xwin[:], identity=ident[:])
                        selfT = winp.tile([128, 128], F32, tag="selfT")
                        nc.vector.tensor_copy(out=selfT[:], in_=stp[:])
                    else:
                        selfT = h1T[:, rows0:rows0 + 128]

                    # ---- out_T = relu(W_top^T @ selfT + W_bot^T @ neighT + b) ----
                    op = psC.tile([128, 128], F32, tag="op")
                    nc.tensor.matmul(op[:], lhsT=wt[:], rhs=(selfT[:] if li == 0 else selfT),
                                     start=True, stop=False)
                    nc.tensor.matmul(op[:], lhsT=wb[:], rhs=neighT[:],
                                     start=False, stop=True)
                    if li == 0:
                        hT_dst = h1T[:, rows0:rows0 + 128]
                    else:
                        hT_t = winp.tile([128, 128], F32, tag="hTout")
                        hT_dst = hT_t[:]
                    nc.scalar.activation(
                        out=hT_dst, in_=op[:],
                        func=mybir.ActivationFunctionType.Relu,
                        bias=bs[:, :1], scale=1.0,
                    )
                    # ---- row-major copy for gather table / output ----
                    rtp = psB.tile([128, 128], F32, tag="tp")
                    nc.tensor.transpose(out=rtp[:], in_=hT_dst, identity=ident[:])
                    h_rm = winp.tile([128, 128], F32, tag="hrm")
                    nc.vector.tensor_copy(out=h_rm[:], in_=rtp[:])
                    if li == 0:
                        nc.sync.dma_start(
                            out=h1loc[rows0:rows0 + nrows, :], in_=h_rm[:nrows, :])
                    else:
                        nc.sync.dma_start(
                            out=out[rows0:rows0 + nrows, 0:D], in_=h_rm[:nrows, :])

            layer(0, xfull, w1t, w1b, b1s)
            nc.gpsimd.collective_compute(
                "AllGather", mybir.AluOpType.bypass,
                replica_groups=[list(range(NCORES))],
                ins=[h1loc.opt()], outs=[h1full.opt()],
            )
            layer(1, h1full, w2t, w2b, b2s)

    nc.compile()
    return nc


def _prep_core(c, x, edge_src, edge_dst, num_neighbors, K):
    lo = c * NLOC
    m = (edge_dst >= lo) & (edge_dst < lo + NLOC)
    es = edge_src[m].astype(np.int64)
    ed = (edge_dst[m].astype(np.int64) - lo)
    w = ed >> 7
    order = np.argsort(w, kind="stable")
    es, ed, w = es[order], ed[order], w[order]
    counts = np.bincount(w, minlength=NWIN)
    starts = np.cumsum(counts) - counts
    pos = np.arange(len(w)) - np.repeat(starts, counts)
    srcs = np.zeros((NWIN, K * 128), np.int32)
    dstr = np.full((NWIN, K * 128), PAD_DST, np.float32)
    srcs[w, pos] = es
    dstr[w, pos] = (ed & 127).astype(np.float32)
    srcs_sb = np.ascontiguousarray(
        srcs.reshape(NWIN, K, 128).transpose(2, 0, 1).reshape(128, NWIN * K))
    dstr_sb = np.ascontiguousarray(
        dstr.reshape(NWIN, K, 128).transpose(2, 0, 1).reshape(128, NWIN * K))
    nnb = np.ones((NPAD,), np.float32)
    nnb[:NLOC] = num_neighbors[lo:lo + NLOC]
    nnbt = np.ascontiguousarray(nnb.reshape(NWIN, 128).T)
    xl = np.zeros((NPAD, D), np.float32)
    xl[:NLOC] = x[lo:lo + NLOC]
    return srcs_sb, dstr_sb, nnbt, xl


def kernel(x, edge_src, edge_dst, num_neighbors, W1, b1, W2, b2):
    x = np.ascontiguousarray(np.asarray(x, dtype=np.float32))
    edge_src = np.asarray(edge_src, dtype=np.int32)
    edge_dst = np.asarray(edge_dst, dtype=np.int32)
    num_neighbors = np.asarray(num_neighbors, dtype=np.float32)
    W1 = np.ascontiguousarray(np.asarray(W1, dtype=np.float32))
    W2 = np.ascontiguousarray(np.asarray(W2, dtype=np.float32))
    b1 = np.asarray(b1, dtype=np.float32).reshape(D, 1)
    b2 = np.asarray(b2, dtype=np.float32).reshape(D, 1)

    # fixed chunk count across cores/windows (one SPMD program)
    core_id = edge_dst.astype(np.int64) // NLOC
    win_id = core_id * NWIN + ((edge_dst.astype(np.int64) % NLOC) >> 7)
    max_cnt = np.bincount(win_id, minlength=NCORES * NWIN).max()
    K = int((max_cnt + 127) // 128)

    if K not in _cache:
        _cache[K] = _build(K)
    nc = _cache[K]

    in_maps = []
    for c in range(NCORES):
        srcs_sb, dstr_sb, nnbt, xl = _prep_core(
            c, x, edge_src, edge_dst, num_neighbors, K)
        in_maps.append({
            "xloc": xl, "xfull": x, "srcs": srcs_sb, "dstr": dstr_sb,
            "nnbt": nnbt, "w1": W1, "b1": b1, "w2": W2, "b2": b2,
        })

    import os
    trace = os.environ.get("GNN_TRACE") == "1"
    res = run_bass_kernel_spmd(nc, in_maps, list(range(NCORES)), trace=trace)
    global last_results
    last_results = res
    if trace and res.exec_time_ns is not None:
        print(f"kernel exec_time_ns: {res.exec_time_ns}")
    return np.concatenate([res.results[c]["out"] for c in range(NCORES)], axis=0)
